# revision 41
# baseline (speedup 1.0000x reference)
"""Trainium2 Bass kernel for the CPA block (sparse/efficient attention).

Strategy
--------
Data parallel over batch: B=128 -> 16 batch elements per NeuronCore, all
parameters replicated (folded on host into a handful of small matrices).

Engine balance vs the original version:
  - LayerNorm stats via *segmented* bn_stats (one instr per <=4 tiles) with
    the mean/var combine done arithmetically on DVE (replaces per-tile
    bn_stats+bn_aggr pairs).
  - LayerNorm normalize runs on GpSimd (was idle; 1-input ops are line rate).
  - k/v biases: v-bias preloaded into PSUM via a K=1 rank-1 matmul
    (ones[1,128] x bias_row); k-bias folded multiplicatively into exp(k)
    with a precomputed exp(bias) table (gpsimd multiply).
  - attention output + FFN second layer computed tokens-first (stationary =
    activation tiles), eliminating the attn^T / h^T PE transposes; rp_b and
    ff2_b biases preloaded into the PSUM accumulators via rank-1 matmuls.
  - residual stream x kept in bf16 (halves SBUF, tolerance is 2e-2).
  - per-element emission is software-pipelined (next element's loads /
    LN / transposes are emitted ahead of the current element's attention)
    so the PE queue always has independent work.
"""

import os

import ml_dtypes
import numpy as np

NB = 16  # batch elements per core
USE_XBAR = bool(int(os.environ.get("KERNEL_XBAR", "1")))
NCORES = 8
EPS = 1e-5
N2, N3, N4, D, MLP = 1024, 256, 64, 128, 512
T2, T3 = N2 // 128, N3 // 128

_PROGRAM = None
LAST_RESULTS = None


def _build_program(nb=NB):
    from contextlib import ExitStack

    import concourse.bacc as bacc
    import concourse.mybir as mybir
    import concourse.tile as tile

    f32 = mybir.dt.float32
    bf16 = mybir.dt.bfloat16
    A = mybir.ActivationFunctionType
    Alu = mybir.AluOpType
    X = mybir.AxisListType.X

    # Restrict the activation-table menu so the load-insertion pass picks the
    # combined natural_log+exp set (serves Ln, Exp, Copy in pass 1) and the
    # gelu set (Gelu, Copy in pass 2) instead of thrashing between the
    # single-function sets.
    class _Bacc(bacc.Bacc):
        _ACT_SETS = {"natural_log_exp_and_others", "gelu_and_others"}

        def insert_act_table_loads(self):
            import bass_rust as _bass_rust

            from concourse.hw_specs import get_activation_tables

            has_activation = any(
                isinstance(i, mybir.InstActivation)
                for b in self.main_func.blocks
                for i in b.instructions
            )
            if not has_activation:
                return
            tables = [
                (name, (fns if name in self._ACT_SETS else set()))
                for name, fns in get_activation_tables(self.m.arch).items()
            ]
            _bass_rust.insert_act_table_loads(self, tables)

    nc = _Bacc("TRN2", target_bir_lowering=False, debug=False)

    def din(name, shape, dt=f32):
        return nc.dram_tensor(name, shape, dt, kind="ExternalInput").ap()

    f2d = din("f2", [nb, N2, D])
    f3d = din("f3", [nb, N3, D])
    f4d = din("f4", [nb, N4, D])
    wq1d = din("wq1", [D, D], bf16)
    wq2d = din("wq2", [D, D], bf16)
    wk1d = din("wk1", [D, D], bf16)
    wv1d = din("wv1", [D, D], bf16)
    wk2d = din("wk2", [D, D], bf16)
    wv2d = din("wv2", [D, D], bf16)
    wrpd = din("wrp", [2, D, D], bf16)
    wff1d = din("wff1", [D, MLP], bf16)
    wff2d = din("wff2", [4, D, D], bf16)
    bq1td = din("bq1t", [D, N2], bf16)
    bq2td = din("bq2t", [D, N2], bf16)
    ebk3d = din("ebk3", [128, T3, 128], bf16)
    ebk4d = din("ebk4", [N4, 128], bf16)
    bv3repd = din("bv3rep", [1, 256], bf16)
    bv4repd = din("bv4rep", [1, 128], bf16)
    rpbrepd = din("rpbrep", [1, 512], bf16)
    ff2bcold = din("ff2bcol", [1, 128], bf16)
    ones512d = din("ones512", [1, 512], bf16)
    ones1d = din("ones1", [1, 128], bf16)
    ff1bd = din("ff1b", [D, 4])
    identd = din("ident", [128, 128], bf16)
    outd = nc.dram_tensor("out", [nb, N2, D], f32, kind="ExternalOutput").ap()

    with tile.TileContext(nc) as tc, ExitStack() as ctx:
        consts = ctx.enter_context(tc.tile_pool(name="consts", bufs=1))
        state = ctx.enter_context(tc.tile_pool(name="state", bufs=1))
        ld = ctx.enter_context(tc.tile_pool(name="ld", bufs=3))
        work = ctx.enter_context(tc.tile_pool(name="work", bufs=6))
        work2 = ctx.enter_context(tc.tile_pool(name="work2", bufs=4))
        workp2 = ctx.enter_context(tc.tile_pool(name="workp2", bufs=2))
        small = ctx.enter_context(tc.tile_pool(name="small", bufs=6))
        pst = ctx.enter_context(tc.tile_pool(name="pst", bufs=2, space="PSUM"))
        psq = ctx.enter_context(tc.tile_pool(name="psq", bufs=2, space="PSUM"))
        pssm = ctx.enter_context(tc.tile_pool(name="pssm", bufs=2, space="PSUM"))
        psout = ctx.enter_context(tc.tile_pool(name="psout", bufs=2, space="PSUM"))

        def cload(name, shape, dt, src):
            t = consts.tile(shape, dt, name=name)
            nc.sync.dma_start(t, src)
            return t

        ident = cload("ident_sb", [128, 128], bf16, identd)
        wq1 = cload("wq1_sb", [D, D], bf16, wq1d)
        wq2 = cload("wq2_sb", [D, D], bf16, wq2d)
        wk1 = cload("wk1_sb", [D, D], bf16, wk1d)
        wv1 = cload("wv1_sb", [D, D], bf16, wv1d)
        wk2 = cload("wk2_sb", [D, D], bf16, wk2d)
        wv2 = cload("wv2_sb", [D, D], bf16, wv2d)
        wrp0 = cload("wrp0_sb", [D, D], bf16, wrpd[0])
        wrp1 = cload("wrp1_sb", [D, D], bf16, wrpd[1])
        bv3rep = cload("bv3rep_sb", [1, 256], bf16, bv3repd)
        bv4rep = cload("bv4rep_sb", [1, 128], bf16, bv4repd)
        rpbrep = cload("rpbrep_sb", [1, 512], bf16, rpbrepd)
        ff2bcol = cload("ff2bcol_sb", [1, 128], bf16, ff2bcold)
        ones512 = cload("ones512_sb", [1, 512], bf16, ones512d)
        ones1 = cload("ones1_sb", [1, 128], bf16, ones1d)
        ff1b = cload("ff1b_sb", [D, 4], f32, ff1bd)
        ebk4 = cload("ebk4_sb", [N4, 128], bf16, ebk4d)
        ebk3 = cload("ebk3_sb", [128, T3, 128], bf16, ebk3d)
        bq1t = cload("bq1t_sb", [D, N2], bf16, bq1td)
        bq2t = cload("bq2t_sb", [D, N2], bf16, bq2td)
        wff1 = cload("wff1_sb", [D, MLP], bf16, wff1d)
        wff2 = consts.tile([128, 4, 128], bf16, name="wff2_sb")
        nc.sync.dma_start(wff2, wff2d.rearrange("j k m -> k j m"))

        x_all = state.tile([128, nb, T2, 128], bf16, name="x_all")
        zxT_all = state.tile([128, nb * N2], bf16, name="zxT_all")
        gm32 = state.tile([128, 2, 128], bf16, name="gm32")
        gm42 = state.tile([128, 2, 128], bf16, name="gm42")
        nc.vector.memset(gm32, 0)
        nc.vector.memset(gm42, 0)

        def bn_stats_raw(out_ap, in_ap):
            # bn_stats with UNOPTIMIZED APs: preserves the element streaming
            # order (the even/odd accumulator split is positional).
            ve = nc.vector
            return ve.add_instruction(mybir.InstBNStats(
                name=nc.get_next_instruction_name(),
                ins=[ve.lower_ap(in_ap, opt=False)],
                outs=[ve.lower_ap(out_ap, opt=False)],
            ))

        def ln_stats(src, ntiles, npart, tag, pool, want_nmb=True):
            """src [npart, ntiles, 128] -> (st, rstd).

            One bn_stats per PAIR of tiles: the input AP interleaves the two
            tiles element-wise (d-major, tile-minor), so the instruction's
            even accumulator sees tile 2k and the odd one tile 2k+1:
              st[:, k] = [128, mean(t2k), 128*var(t2k),
                          128, mean(t2k+1), 128*var(t2k+1)]
            mean for tile t lives at st[:, t//2, 1+3*(t%2)];
            rstd[:, t%2, t//2] = 1/sqrt(var_t + eps).
            """
            assert ntiles % 2 == 0
            npairs = ntiles // 2
            st = pool.tile([npart, npairs, 6], f32, tag=f"st_{tag}", name="st")
            for k in range(npairs):
                bn_stats_raw(
                    st[:, k, :],
                    src[:, 2 * k : 2 * k + 2, :].rearrange("p t d -> p d t"),
                )
            rstd = pool.tile([npart, 2, npairs], f32, tag=f"r_{tag}", name="rstd")
            nc.vector.tensor_scalar(rstd[:, 0, :], st[:, :, 2:3], 1.0 / 128.0,
                                    EPS, Alu.mult, Alu.add)
            nc.vector.tensor_scalar(rstd[:, 1, :], st[:, :, 5:6], 1.0 / 128.0,
                                    EPS, Alu.mult, Alu.add)
            # rstd = exp(-0.5*ln(var+eps))
            nc.scalar.activation(rstd, rstd, A.Ln)
            nc.scalar.activation(rstd, rstd, A.Exp, scale=-0.5)
            if not want_nmb:
                return st, rstd, None
            # compact the per-tile means (ACT copy - keeps this off the
            # saturated DVE) for the broadcast normalize
            mc = pool.tile([npart, 2, npairs], f32, tag=f"nm_{tag}", name="mc")
            for j in range(2):
                nc.scalar.activation(
                    mc[:, j, :], st[:, :, 1 + 3 * j : 2 + 3 * j], A.Copy)
            return st, rstd, mc

        def ln_stats_one(src, npart, tag, pool):
            """Single-tile layernorm stats (classic bn_stats + bn_aggr)."""
            st = pool.tile([npart, 6], f32, tag=f"st_{tag}", name="st")
            nc.vector.bn_stats(st, src)
            mv = pool.tile([npart, 2], f32, tag=f"mv_{tag}", name="mv")
            nc.vector.bn_aggr(mv, st)
            rstd = pool.tile([npart, 1], f32, tag=f"r_{tag}", name="rstd")
            nc.vector.tensor_scalar(rstd, mv[:, 1:2], EPS, None, Alu.add)
            nc.scalar.activation(rstd, rstd, A.Ln)
            nc.scalar.activation(rstd, rstd, A.Exp, scale=-0.5)
            return mv, rstd, None

        def bcast_norm(eng, dst, src, rstd, nmb, ntiles):
            """dst = src*rstd + nmb via two broadcast tensor_tensors."""
            np_ = ntiles // 2
            s4v = src.rearrange("p (k j) d -> p k j d", j=2)
            d4v = dst.rearrange("p (k j) d -> p k j d", j=2)
            r4v = rstd.rearrange("p j k -> p k j").unsqueeze(3).broadcast_to(
                [128, np_, 2, 128])
            n4v = nmb.rearrange("p j k -> p k j").unsqueeze(3).broadcast_to(
                [128, np_, 2, 128])
            eng.tensor_tensor(d4v, s4v, n4v, Alu.subtract)
            eng.tensor_tensor(d4v, d4v, r4v, Alu.mult)

        def transpose_to(dst, z, ntiles, mode):
            """z: [128, ntiles, 128] bf16 -> dst [128, ntiles*128] bf16.

            mode: "pe" = PE transpose + DVE copy; "xbar" = DMA xbar spread
            across the two HWDGE queues (sync + scalar).
            """
            if mode == "xbar":
                for i in range(ntiles):
                    nc.sync.dma_start_transpose(
                        dst[:, i * 128 : (i + 1) * 128], z[:, i, :])
                return
            pe_tiles = ntiles if mode == "pe" else ntiles // 2
            for i in range(pe_tiles, ntiles):
                nc.sync.dma_start_transpose(
                    dst[:, i * 128 : (i + 1) * 128], z[:, i, :])
            for i in range(0, pe_tiles, 4):
                j = min(i + 4, pe_tiles)
                ps = pst.tile([128, 512], bf16, tag="pst", name="ps")
                for q in range(i, j):
                    nc.tensor.transpose(
                        ps[:, (q - i) * 128 : (q - i + 1) * 128], z[:, q, :], ident
                    )
                nc.scalar.activation(dst[:, i * 128 : j * 128], ps[:, : (j - i) * 128], A.Copy)

        # per-element stage S1: DMA loads + LN stats + normalize
        def stage1(b):
            f2t = ld.tile([128, T2, 128], f32, tag="f2t", name="f2t")
            nc.sync.dma_start(f2t, f2d[b].rearrange("(t p) d -> p t d", p=128))
            f3t = ld.tile([128, T3, 128], f32, tag="f3t", name="f3t")
            nc.sync.dma_start(f3t, f3d[b].rearrange("(t p) d -> p t d", p=128))
            f4t = ld.tile([N4, 1, 128], f32, tag="f4t", name="f4t")
            nc.sync.dma_start(f4t[:, 0, :], f4d[b])

            st2, r2, nm2 = ln_stats(f2t, T2, 128, "t2", small)
            st3, r3, nm3 = ln_stats(f3t, T3, 128, "t3", small, want_nmb=False)
            mv4, r4, nm4 = ln_stats_one(f4t[:, 0, :], N4, "t4", small)
            z2 = work.tile([128, T2, 128], bf16, tag="z2", name="z2")
            bcast_norm(nc.gpsimd, z2, f2t, r2, nm2, T2)
            z3 = work.tile([128, T3, 128], bf16, tag="z3", name="z3")
            for t in range(T3):
                k, j = t // 2, t % 2
                nc.vector.tensor_scalar(
                    z3[:, t, :], f3t[:, t, :],
                    st3[:, k, 1 + 3 * j : 2 + 3 * j], r3[:, j, k : k + 1],
                    Alu.subtract, Alu.mult,
                )
            z4 = work.tile([N4, 1, 128], bf16, tag="z4", name="z4")
            nc.vector.tensor_scalar(
                z4[:, 0, :], f4t[:, 0, :], mv4[:, 0:1], r4,
                Alu.subtract, Alu.mult,
            )
            f2bf = work.tile([128, T2, 128], bf16, tag="f2bf", name="f2bf")
            nc.vector.tensor_copy(f2bf, f2t)
            return f2bf, z2, z3, z4

        # stage S2: transposes to channels-first
        def stage2(b, z2, z3, z4):
            z2T = work.tile([128, N2], bf16, tag="z2T", name="z2T")
            transpose_to(z2T, z2, T2, mode="pe")
            z3T = work.tile([128, N3], bf16, tag="z3T", name="z3T")
            transpose_to(z3T, z3, T3, mode="xbar")
            z4T = work.tile([128, N4], bf16, tag="z4T", name="z4T")
            nc.sync.dma_start_transpose(z4T, z4[:, 0, :])
            return z2T, z3T, z4T

        # stage S3: projections, kv, gram, m-matrices
        def stage3(b, z2T, z3T, z4T):
            # ---- q projections + exp (+ free softmax denominators) ----
            eq1 = work2.tile([128, N2], bf16, tag="eq1", name="eq1")
            eq2 = work2.tile([128, N2], bf16, tag="eq2", name="eq2")
            S = small.tile([128, 2, 2], f32, tag="S", name="S")
            for qi, (wq, bqt, eq) in enumerate(((wq1, bq1t, eq1), (wq2, bq2t, eq2))):
                qp0 = psq.tile([128, 512], f32, tag="q", name="qp0")
                qp1 = psq.tile([128, 512], f32, tag="q", name="qp1")
                nc.tensor.matmul(qp0, wq, z2T[:, 0:512], start=True, stop=False)
                nc.tensor.matmul(qp1, wq, z2T[:, 512:1024], start=True, stop=False)
                nc.tensor.matmul(qp0, ident, bqt[:, 0:512], start=False, stop=True)
                nc.tensor.matmul(qp1, ident, bqt[:, 512:1024], start=False, stop=True)
                nc.scalar.activation(eq[:, 0:512], qp0, A.Exp,
                                     accum_out=S[:, qi, 0:1])
                nc.scalar.activation(eq[:, 512:1024], qp1, A.Exp,
                                     accum_out=S[:, qi, 1:2])
            rS = small.tile([128, 2], f32, tag="rS", name="rS")
            nc.vector.tensor_tensor(rS, S[:, :, 0], S[:, :, 1], Alu.add)
            nc.vector.reciprocal(rS, rS)

            # ---- k3/v3 (tokens-first; v-bias preloaded, k-bias via exp table) ----
            kv3p = pssm.tile([128, 512], f32, tag="sm", name="kv3p")
            nc.tensor.matmul(kv3p[:, 256:512], ones1, bv3rep,
                             start=True, stop=False, skip_group_check=True)
            for t in range(T3):
                z3s = z3T[:, t * 128 : (t + 1) * 128]
                nc.tensor.matmul(kv3p[:, t * 128 : (t + 1) * 128], z3s, wk1,
                                 start=True, stop=True, skip_group_check=True)
                nc.tensor.matmul(kv3p[:, 256 + t * 128 : 256 + (t + 1) * 128], z3s,
                                 wv1, start=False, stop=(t == T3 - 1),
                                 skip_group_check=True)
            ek3r = work2.tile([128, T3, 128], bf16, tag="ek3r", name="ek3r")
            nc.scalar.activation(
                ek3r, kv3p[:, 0:256].rearrange("p (t d) -> p t d", t=T3), A.Exp)
            ek3 = work2.tile([128, T3, 128], bf16, tag="ek3", name="ek3")
            nc.vector.tensor_tensor(ek3, ek3r, ebk3, Alu.mult)
            s3 = small.tile([128, T3, 2], f32, tag="s3", name="s3")
            nc.vector.tensor_reduce(
                s3, ek3.rearrange("p t (h e) -> p t h e", h=2), axis=X, op=Alu.add
            )
            nc.vector.reciprocal(s3, s3)
            v3s = work2.tile([128, T3, 128], bf16, tag="v3s", name="v3s")
            nc.vector.tensor_tensor(
                v3s.rearrange("p t (h e) -> p t h e", h=2),
                kv3p[:, 256:512].rearrange("p (t h e) -> p t h e", t=T3, h=2),
                s3.unsqueeze(3).broadcast_to([128, T3, 2, 64]),
                Alu.mult,
            )

            # ---- k4/v4 ----
            kv4p = pssm.tile([N4, 512], f32, tag="sm", name="kv4p")
            nc.tensor.matmul(kv4p[:, 128:256], ones1[:, :N4], bv4rep,
                             start=True, stop=False, skip_group_check=True)
            nc.tensor.matmul(kv4p[:, 0:128], z4T, wk2, start=True, stop=True,
                             skip_group_check=True)
            nc.tensor.matmul(kv4p[:, 128:256], z4T, wv2, start=False, stop=True,
                             skip_group_check=True)
            ek4r = work2.tile([N4, 128], bf16, tag="ek4r", name="ek4r")
            nc.scalar.activation(ek4r, kv4p[:, 0:128], A.Exp)
            ek4 = work2.tile([N4, 128], bf16, tag="ek4", name="ek4")
            nc.vector.tensor_tensor(ek4, ek4r, ebk4, Alu.mult)
            s4 = small.tile([N4, 1, 2], f32, tag="s4", name="s4")
            nc.vector.tensor_reduce(
                s4, ek4.rearrange("p (o h e) -> p o h e", o=1, h=2), axis=X, op=Alu.add
            )
            nc.vector.reciprocal(s4, s4)
            v4s = work2.tile([N4, 128], bf16, tag="v4s", name="v4s")
            nc.vector.tensor_tensor(
                v4s.rearrange("p (h e) -> p h e", h=2),
                kv4p[:, 128:256].rearrange("p (h e) -> p h e", h=2),
                s4[:, 0, :].unsqueeze(2).broadcast_to([N4, 2, 64]),
                Alu.mult,
            )

            # ---- Gram (transposed) + block-diag mask + rp fold ----
            g32p = pssm.tile([128, 128], f32, tag="sm", name="g32p")
            for t in range(T3):
                nc.tensor.matmul(g32p, v3s[:, t, :], ek3[:, t, :],
                                 start=(t == 0), stop=(t == T3 - 1))
            g42p = pssm.tile([128, 128], f32, tag="sm", name="g42p")
            nc.tensor.matmul(g42p, v4s, ek4, start=True, stop=True)

            sl = b % 2
            ms = []
            for gi, (gp, gmt, wrp_, qi) in enumerate(
                ((g32p, gm32, wrp0, 0), (g42p, gm42, wrp1, 1))
            ):
                for h in range(2):
                    nc.vector.tensor_copy(
                        gmt[h * 64 : (h + 1) * 64, sl, h * 64 : (h + 1) * 64],
                        gp[h * 64 : (h + 1) * 64, h * 64 : (h + 1) * 64],
                    )
                mp = pssm.tile([128, 128], f32, tag="sm", name="mp")
                nc.tensor.matmul(mp, gmt[:, sl, :], wrp_,
                                 start=True, stop=True)
                m = work2.tile([128, 128], bf16, tag="m_", name="m_")
                nc.vector.tensor_scalar(m, mp, rS[:, qi : qi + 1], None, Alu.mult)
                ms.append(m)
            return eq1, eq2, ms[0], ms[1]

        # stage S4: attention output, residual, LN4, zx^T
        def stage4(b, f2t, eq1, eq2, m32, m42):
            x_b = x_all[:, b]
            for c in range(2):
                ap_ = psout.tile([128, 512], f32, tag="o", name="ap_")
                nc.tensor.matmul(ap_, ones1, rpbrep, start=True, stop=False,
                                 skip_group_check=True)
                for t in range(4):
                    g = c * 4 + t
                    nc.tensor.matmul(
                        ap_[:, t * 128 : (t + 1) * 128],
                        eq1[:, g * 128 : (g + 1) * 128], m32,
                        start=False, stop=False, skip_group_check=True)
                    nc.tensor.matmul(
                        ap_[:, t * 128 : (t + 1) * 128],
                        eq2[:, g * 128 : (g + 1) * 128], m42,
                        start=False, stop=(t == 3), skip_group_check=True)
                # x = f2 + attn (tokens-first already)
                nc.vector.tensor_tensor(
                    x_b[:, c * 4 : (c + 1) * 4, :].rearrange("p t d -> p (t d)"),
                    f2t[:, c * 4 : (c + 1) * 4, :].rearrange("p t d -> p (t d)"),
                    ap_, Alu.add,
                )

            # ---- LN4 + zx^T ----
            stx, rx, nmx = ln_stats(x_b, T2, 128, "t2x", small)
            zx = work2.tile([128, T2, 128], bf16, tag="zx", name="zx")
            bcast_norm(nc.gpsimd, zx, x_b, rx, nmx, T2)
            transpose_to(zxT_all[:, b * N2 : (b + 1) * N2], zx, T2, mode="mix")

        # pass-2 FFN for one element
        def pass2(b):
            zxT = zxT_all[:, b * N2 : (b + 1) * N2]
            x_b = x_all[:, b]
            out_sb = workp2.tile([128, T2, 128], f32, tag="osb", name="out_sb")
            for c in range(2):
                hp = psout.tile([128, 512], f32, tag="o", name="hp")
                nc.tensor.matmul(hp, ff2bcol, ones512, start=True, stop=False,
                                 skip_group_check=True)
                gps = [None] * 4
                gps[0] = psq.tile([128, 512], f32, tag="q", name="gp")
                nc.tensor.matmul(gps[0], wff1[:, 0:128],
                                 zxT[:, c * 512 : (c + 1) * 512],
                                 start=True, stop=True)
                for j in range(4):
                    gj = workp2.tile([128, 512], bf16, tag="gj", name="gj")
                    nc.scalar.activation(gj, gps[j], A.Gelu, bias=ff1b[:, j : j + 1])
                    if j < 3:
                        gps[j + 1] = psq.tile([128, 512], f32, tag="q", name="gp")
                        nc.tensor.matmul(gps[j + 1],
                                         wff1[:, (j + 1) * 128 : (j + 2) * 128],
                                         zxT[:, c * 512 : (c + 1) * 512],
                                         start=True, stop=True)
                    nc.tensor.matmul(hp, wff2[:, j, :], gj,
                                     start=False, stop=(j == 3),
                                     skip_group_check=True)
                hcf = workp2.tile([128, 512], bf16, tag="hcf", name="hcf")
                nc.vector.tensor_copy(hcf, hp)
                ps = pst.tile([128, 512], bf16, tag="pst", name="psh")
                for t in range(4):
                    nc.tensor.transpose(
                        ps[:, t * 128 : (t + 1) * 128],
                        hcf[:, t * 128 : (t + 1) * 128], ident)
                nc.vector.tensor_tensor(
                    out_sb[:, c * 4 : (c + 1) * 4, :].rearrange("p t d -> p (t d)"),
                    x_b[:, c * 4 : (c + 1) * 4, :].rearrange("p t d -> p (t d)"),
                    ps, Alu.add,
                )
            nc.sync.dma_start(outd[b].rearrange("(t p) d -> p t d", p=128), out_sb)

        # ---------------- pipelined emission ----------------
        # pass-1 stages with a 3-deep skew; pass-2 emitted in 4-element
        # batches as soon as its inputs are complete, so the scheduler can
        # fill PE gaps (and keep HAM warm) with dense FFN matmuls while
        # bounding ACT-table swaps.
        npair = nb // 2
        s1o = {}
        s12 = {}
        s3o = {}
        for i in range(npair + 2):
            if i < npair:
                for b in (2 * i, 2 * i + 1):
                    s1o[b] = stage1(b)
            if 0 <= i - 1 < npair:
                for b in (2 * (i - 1), 2 * (i - 1) + 1):
                    s3o[b] = stage3(b, *s12[b][1])
            if 0 <= i - 2 < npair:
                for b in (2 * (i - 2), 2 * (i - 2) + 1):
                    stage4(b, s12[b][0], *s3o.pop(b))
                    del s12[b]
            if i < npair:
                for b in (2 * i, 2 * i + 1):
                    f2bf, z2, z3, z4 = s1o.pop(b)
                    s12[b] = (f2bf, stage2(b, z2, z3, z4))
        for b2 in range(nb):
            pass2(b2)

    nc.compile()
    return nc


def _get_program():
    global _PROGRAM
    if _PROGRAM is None:
        _PROGRAM = _build_program(NB)
    return _PROGRAM


def _prepare_params(inputs):
    bf = ml_dtypes.bfloat16
    g = {k: np.asarray(v, np.float32) for k, v in inputs.items()
         if k not in ("f2", "f3", "f4")}
    pe2, pe3, pe4 = g["pe2"][0], g["pe3"][0], g["pe4"][0]

    def fold_w(ln_w, w):
        return np.ascontiguousarray(ln_w[:, None] * w).astype(bf)

    def fold_bt(ln_b, pe, w, b):
        return np.ascontiguousarray(((ln_b[None, :] + pe) @ w + b[None, :]).T).astype(bf)

    p = {}
    p["wq1"] = fold_w(g["ln1_w"], g["q1_w"])
    p["wq2"] = fold_w(g["ln1_w"], g["q2_w"])
    p["wk1"] = fold_w(g["ln2_w"], g["k1_w"])
    p["wv1"] = fold_w(g["ln2_w"], g["v1_w"])
    p["wk2"] = fold_w(g["ln3_w"], g["k2_w"])
    p["wv2"] = fold_w(g["ln3_w"], g["v2_w"])
    p["bq1t"] = fold_bt(g["ln1_b"], pe2, g["q1_w"], g["q1_b"])
    p["bq2t"] = fold_bt(g["ln1_b"], pe2, g["q2_w"], g["q2_b"])
    bk3 = (g["ln2_b"][None, :] + pe3) @ g["k1_w"] + g["k1_b"][None, :]
    bk4 = (g["ln3_b"][None, :] + pe4) @ g["k2_w"] + g["k2_b"][None, :]
    p["ebk3"] = np.ascontiguousarray(
        np.exp(bk3).reshape(T3, 128, 128).transpose(1, 0, 2)).astype(bf)
    p["ebk4"] = np.ascontiguousarray(np.exp(bk4)).astype(bf)
    bv3row = g["ln2_b"] @ g["v1_w"] + g["v1_b"]
    bv4row = g["ln3_b"] @ g["v2_w"] + g["v2_b"]
    p["bv3rep"] = np.ascontiguousarray(np.tile(bv3row, 2)[None, :]).astype(bf)
    p["bv4rep"] = np.ascontiguousarray(bv4row[None, :]).astype(bf)
    p["rpbrep"] = np.ascontiguousarray(np.tile(g["rp_b"], 4)[None, :]).astype(bf)
    p["wrp"] = np.ascontiguousarray(g["rp_w"].reshape(2, D, D)).astype(bf)
    p["wff1"] = fold_w(g["ln4_w"], g["ff1_w"])
    bff1 = g["ln4_b"] @ g["ff1_w"] + g["ff1_b"]
    p["ff1b"] = np.ascontiguousarray(bff1.reshape(4, 128).T).astype(np.float32)
    p["wff2"] = np.ascontiguousarray(g["ff2_w"].reshape(4, 128, D)).astype(bf)
    p["ff2bcol"] = np.ascontiguousarray(g["ff2_b"][None, :]).astype(bf)
    p["ones512"] = np.ones((1, 512), np.float32).astype(bf)
    p["ones1"] = np.ones((1, 128), np.float32).astype(bf)
    p["ident"] = np.eye(128, dtype=np.float32).astype(bf)
    return p


def kernel(**inputs):
    global LAST_RESULTS
    from concourse import bass_utils

    f2 = np.ascontiguousarray(np.asarray(inputs["f2"], np.float32))
    f3 = np.ascontiguousarray(np.asarray(inputs["f3"], np.float32))
    f4 = np.ascontiguousarray(np.asarray(inputs["f4"], np.float32))
    params = _prepare_params(inputs)
    nc = _get_program()

    in_maps = []
    for c in range(NCORES):
        m = dict(params)
        sl = slice(c * NB, (c + 1) * NB)
        m["f2"] = f2[sl]
        m["f3"] = f3[sl]
        m["f4"] = f4[sl]
        in_maps.append(m)

    res = bass_utils.run_bass_kernel_spmd(
        nc, in_maps, list(range(NCORES)),
        trace=bool(int(os.environ.get("KERNEL_TRACE", "0"))),
    )
    LAST_RESULTS = res
    out = np.concatenate([r["out"] for r in res.results], axis=0)
    return np.ascontiguousarray(out.astype(np.float32))


# revision 42
# speedup vs baseline: 1.0091x; 1.0091x over previous
"""Trainium2 Bass kernel for the CPA block (sparse/efficient attention).

Strategy
--------
Data parallel over batch: B=128 -> 16 batch elements per NeuronCore, all
parameters replicated (folded on host into a handful of small matrices).

Engine balance vs the original version:
  - LayerNorm stats via *segmented* bn_stats (one instr per <=4 tiles) with
    the mean/var combine done arithmetically on DVE (replaces per-tile
    bn_stats+bn_aggr pairs).
  - LayerNorm normalize runs on GpSimd (was idle; 1-input ops are line rate).
  - k/v biases: v-bias preloaded into PSUM via a K=1 rank-1 matmul
    (ones[1,128] x bias_row); k-bias folded multiplicatively into exp(k)
    with a precomputed exp(bias) table (gpsimd multiply).
  - attention output + FFN second layer computed tokens-first (stationary =
    activation tiles), eliminating the attn^T / h^T PE transposes; rp_b and
    ff2_b biases preloaded into the PSUM accumulators via rank-1 matmuls.
  - residual stream x kept in bf16 (halves SBUF, tolerance is 2e-2).
  - per-element emission is software-pipelined (next element's loads /
    LN / transposes are emitted ahead of the current element's attention)
    so the PE queue always has independent work.
"""

import os

import ml_dtypes
import numpy as np

NB = 16  # batch elements per core
USE_XBAR = bool(int(os.environ.get("KERNEL_XBAR", "1")))
NCORES = 8
EPS = 1e-5
N2, N3, N4, D, MLP = 1024, 256, 64, 128, 512
T2, T3 = N2 // 128, N3 // 128

_PROGRAM = None
LAST_RESULTS = None


def _build_program(nb=NB):
    from contextlib import ExitStack

    import concourse.bacc as bacc
    import concourse.mybir as mybir
    import concourse.tile as tile

    f32 = mybir.dt.float32
    bf16 = mybir.dt.bfloat16
    A = mybir.ActivationFunctionType
    Alu = mybir.AluOpType
    X = mybir.AxisListType.X

    # Restrict the activation-table menu so the load-insertion pass picks the
    # combined natural_log+exp set (serves Ln, Exp, Copy in pass 1) and the
    # gelu set (Gelu, Copy in pass 2) instead of thrashing between the
    # single-function sets.
    class _Bacc(bacc.Bacc):
        _ACT_SETS = {"natural_log_exp_and_others", "gelu_and_others"}

        def insert_act_table_loads(self):
            import bass_rust as _bass_rust

            from concourse.hw_specs import get_activation_tables

            has_activation = any(
                isinstance(i, mybir.InstActivation)
                for b in self.main_func.blocks
                for i in b.instructions
            )
            if not has_activation:
                return
            tables = [
                (name, (fns if name in self._ACT_SETS else set()))
                for name, fns in get_activation_tables(self.m.arch).items()
            ]
            _bass_rust.insert_act_table_loads(self, tables)

    nc = _Bacc("TRN2", target_bir_lowering=False, debug=False)

    def din(name, shape, dt=f32):
        return nc.dram_tensor(name, shape, dt, kind="ExternalInput").ap()

    f2d = din("f2", [nb, N2, D])
    f3d = din("f3", [nb, N3, D])
    f4d = din("f4", [nb, N4, D])
    wq1d = din("wq1", [D, D], bf16)
    wq2d = din("wq2", [D, D], bf16)
    wk1d = din("wk1", [D, D], bf16)
    wv1d = din("wv1", [D, D], bf16)
    wk2d = din("wk2", [D, D], bf16)
    wv2d = din("wv2", [D, D], bf16)
    wrpd = din("wrp", [2, D, D], bf16)
    wff1d = din("wff1", [D, MLP], bf16)
    wff2d = din("wff2", [4, D, D], bf16)
    bq1td = din("bq1t", [D, N2], bf16)
    bq2td = din("bq2t", [D, N2], bf16)
    ebk3d = din("ebk3", [128, T3, 128], bf16)
    ebk4d = din("ebk4", [N4, 128], bf16)
    bv3repd = din("bv3rep", [1, 256], bf16)
    bv4repd = din("bv4rep", [1, 128], bf16)
    rpbrepd = din("rpbrep", [1, 512], bf16)
    ff2bcold = din("ff2bcol", [1, 128], bf16)
    ones512d = din("ones512", [1, 512], bf16)
    ones1d = din("ones1", [1, 128], bf16)
    ff1bd = din("ff1b", [D, 4])
    identd = din("ident", [128, 128], bf16)
    outd = nc.dram_tensor("out", [nb, N2, D], f32, kind="ExternalOutput").ap()

    with tile.TileContext(nc) as tc, ExitStack() as ctx:
        consts = ctx.enter_context(tc.tile_pool(name="consts", bufs=1))
        state = ctx.enter_context(tc.tile_pool(name="state", bufs=1))
        ld = ctx.enter_context(tc.tile_pool(name="ld", bufs=3))
        work = ctx.enter_context(tc.tile_pool(name="work", bufs=6))
        work2 = ctx.enter_context(tc.tile_pool(name="work2", bufs=4))
        workp2 = ctx.enter_context(tc.tile_pool(name="workp2", bufs=2))
        small = ctx.enter_context(tc.tile_pool(name="small", bufs=6))
        pst = ctx.enter_context(tc.tile_pool(name="pst", bufs=2, space="PSUM"))
        psq = ctx.enter_context(tc.tile_pool(name="psq", bufs=2, space="PSUM"))
        pssm = ctx.enter_context(tc.tile_pool(name="pssm", bufs=2, space="PSUM"))
        psout = ctx.enter_context(tc.tile_pool(name="psout", bufs=2, space="PSUM"))

        def cload(name, shape, dt, src):
            t = consts.tile(shape, dt, name=name)
            nc.sync.dma_start(t, src)
            return t

        ident = cload("ident_sb", [128, 128], bf16, identd)
        wq1 = cload("wq1_sb", [D, D], bf16, wq1d)
        wq2 = cload("wq2_sb", [D, D], bf16, wq2d)
        wk1 = cload("wk1_sb", [D, D], bf16, wk1d)
        wv1 = cload("wv1_sb", [D, D], bf16, wv1d)
        wk2 = cload("wk2_sb", [D, D], bf16, wk2d)
        wv2 = cload("wv2_sb", [D, D], bf16, wv2d)
        wrp0 = cload("wrp0_sb", [D, D], bf16, wrpd[0])
        wrp1 = cload("wrp1_sb", [D, D], bf16, wrpd[1])
        bv3rep = cload("bv3rep_sb", [1, 256], bf16, bv3repd)
        bv4rep = cload("bv4rep_sb", [1, 128], bf16, bv4repd)
        rpbrep = cload("rpbrep_sb", [1, 512], bf16, rpbrepd)
        ff2bcol = cload("ff2bcol_sb", [1, 128], bf16, ff2bcold)
        ones512 = cload("ones512_sb", [1, 512], bf16, ones512d)
        ones1 = cload("ones1_sb", [1, 128], bf16, ones1d)
        ff1b = cload("ff1b_sb", [D, 4], f32, ff1bd)
        ebk4 = cload("ebk4_sb", [N4, 128], bf16, ebk4d)
        ebk3 = cload("ebk3_sb", [128, T3, 128], bf16, ebk3d)
        bq1t = cload("bq1t_sb", [D, N2], bf16, bq1td)
        bq2t = cload("bq2t_sb", [D, N2], bf16, bq2td)
        wff1 = cload("wff1_sb", [D, MLP], bf16, wff1d)
        wff2 = consts.tile([128, 4, 128], bf16, name="wff2_sb")
        nc.sync.dma_start(wff2, wff2d.rearrange("j k m -> k j m"))

        x_all = state.tile([128, nb, T2, 128], bf16, name="x_all")
        zxT_all = state.tile([128, nb * N2], bf16, name="zxT_all")
        gm32 = state.tile([128, 2, 128], bf16, name="gm32")
        gm42 = state.tile([128, 2, 128], bf16, name="gm42")
        nc.vector.memset(gm32, 0)
        nc.vector.memset(gm42, 0)

        def bn_stats_raw(out_ap, in_ap):
            # bn_stats with UNOPTIMIZED APs: preserves the element streaming
            # order (the even/odd accumulator split is positional).
            ve = nc.vector
            return ve.add_instruction(mybir.InstBNStats(
                name=nc.get_next_instruction_name(),
                ins=[ve.lower_ap(in_ap, opt=False)],
                outs=[ve.lower_ap(out_ap, opt=False)],
            ))

        def ln_stats(src, ntiles, npart, tag, pool, want_nmb=True):
            """src [npart, ntiles, 128] -> (st, rstd).

            One bn_stats per PAIR of tiles: the input AP interleaves the two
            tiles element-wise (d-major, tile-minor), so the instruction's
            even accumulator sees tile 2k and the odd one tile 2k+1:
              st[:, k] = [128, mean(t2k), 128*var(t2k),
                          128, mean(t2k+1), 128*var(t2k+1)]
            mean for tile t lives at st[:, t//2, 1+3*(t%2)];
            rstd[:, t%2, t//2] = 1/sqrt(var_t + eps).
            """
            assert ntiles % 2 == 0
            npairs = ntiles // 2
            st = pool.tile([npart, npairs, 6], f32, tag=f"st_{tag}", name="st")
            for k in range(npairs):
                bn_stats_raw(
                    st[:, k, :],
                    src[:, 2 * k : 2 * k + 2, :].rearrange("p t d -> p d t"),
                )
            rstd = pool.tile([npart, 2, npairs], f32, tag=f"r_{tag}", name="rstd")
            nc.vector.tensor_scalar(rstd[:, 0, :], st[:, :, 2:3], 1.0 / 128.0,
                                    EPS, Alu.mult, Alu.add)
            nc.vector.tensor_scalar(rstd[:, 1, :], st[:, :, 5:6], 1.0 / 128.0,
                                    EPS, Alu.mult, Alu.add)
            # rstd = exp(-0.5*ln(var+eps))
            nc.scalar.activation(rstd, rstd, A.Ln)
            nc.scalar.activation(rstd, rstd, A.Exp, scale=-0.5)
            if not want_nmb:
                return st, rstd, None
            # nmb = -mean * rstd  (bias for the broadcast normalize)
            nmb = pool.tile([npart, 2, npairs], f32, tag=f"nm_{tag}", name="nmb")
            for j in range(2):
                nc.vector.scalar_tensor_tensor(
                    nmb[:, j, :], st[:, :, 1 + 3 * j : 2 + 3 * j], -1.0,
                    rstd[:, j, :], Alu.mult, Alu.mult)
            return st, rstd, nmb

        def ln_stats_one(src, npart, tag, pool):
            """Single-tile layernorm stats (classic bn_stats + bn_aggr)."""
            st = pool.tile([npart, 6], f32, tag=f"st_{tag}", name="st")
            nc.vector.bn_stats(st, src)
            mv = pool.tile([npart, 2], f32, tag=f"mv_{tag}", name="mv")
            nc.vector.bn_aggr(mv, st)
            rstd = pool.tile([npart, 1], f32, tag=f"r_{tag}", name="rstd")
            nc.vector.tensor_scalar(rstd, mv[:, 1:2], EPS, None, Alu.add)
            nc.scalar.activation(rstd, rstd, A.Ln)
            nc.scalar.activation(rstd, rstd, A.Exp, scale=-0.5)
            return mv, rstd, None

        def bcast_norm(eng, dst, src, rstd, nmb, ntiles):
            """dst = src*rstd + nmb via two broadcast tensor_tensors."""
            np_ = ntiles // 2
            s4v = src.rearrange("p (k j) d -> p k j d", j=2)
            d4v = dst.rearrange("p (k j) d -> p k j d", j=2)
            r4v = rstd.rearrange("p j k -> p k j").unsqueeze(3).broadcast_to(
                [128, np_, 2, 128])
            n4v = nmb.rearrange("p j k -> p k j").unsqueeze(3).broadcast_to(
                [128, np_, 2, 128])
            eng.tensor_tensor(d4v, s4v, r4v, Alu.mult)
            eng.tensor_tensor(d4v, d4v, n4v, Alu.add)

        def transpose_to(dst, z, ntiles, mode):
            """z: [128, ntiles, 128] bf16 -> dst [128, ntiles*128] bf16.

            mode: "pe" = PE transpose + DVE copy; "xbar" = DMA xbar spread
            across the two HWDGE queues (sync + scalar).
            """
            if mode == "xbar":
                for i in range(ntiles):
                    nc.sync.dma_start_transpose(
                        dst[:, i * 128 : (i + 1) * 128], z[:, i, :])
                return
            pe_tiles = ntiles if mode == "pe" else ntiles // 2
            for i in range(pe_tiles, ntiles):
                nc.sync.dma_start_transpose(
                    dst[:, i * 128 : (i + 1) * 128], z[:, i, :])
            for i in range(0, pe_tiles, 4):
                j = min(i + 4, pe_tiles)
                ps = pst.tile([128, 512], bf16, tag="pst", name="ps")
                for q in range(i, j):
                    nc.tensor.transpose(
                        ps[:, (q - i) * 128 : (q - i + 1) * 128], z[:, q, :], ident
                    )
                nc.scalar.activation(dst[:, i * 128 : j * 128], ps[:, : (j - i) * 128], A.Copy)

        # per-element stage S1: DMA loads + LN stats + normalize
        def stage1(b):
            f2t = ld.tile([128, T2, 128], f32, tag="f2t", name="f2t")
            nc.sync.dma_start(f2t, f2d[b].rearrange("(t p) d -> p t d", p=128))
            f3t = ld.tile([128, T3, 128], f32, tag="f3t", name="f3t")
            nc.sync.dma_start(f3t, f3d[b].rearrange("(t p) d -> p t d", p=128))
            f4t = ld.tile([N4, 1, 128], f32, tag="f4t", name="f4t")
            nc.sync.dma_start(f4t[:, 0, :], f4d[b])

            st2, r2, nm2 = ln_stats(f2t, T2, 128, "t2", small)
            st3, r3, nm3 = ln_stats(f3t, T3, 128, "t3", small, want_nmb=False)
            mv4, r4, nm4 = ln_stats_one(f4t[:, 0, :], N4, "t4", small)
            z2 = work.tile([128, T2, 128], bf16, tag="z2", name="z2")
            bcast_norm(nc.gpsimd, z2, f2t, r2, nm2, T2)
            z3 = work.tile([128, T3, 128], bf16, tag="z3", name="z3")
            for t in range(T3):
                k, j = t // 2, t % 2
                nc.vector.tensor_scalar(
                    z3[:, t, :], f3t[:, t, :],
                    st3[:, k, 1 + 3 * j : 2 + 3 * j], r3[:, j, k : k + 1],
                    Alu.subtract, Alu.mult,
                )
            z4 = work.tile([N4, 1, 128], bf16, tag="z4", name="z4")
            nc.vector.tensor_scalar(
                z4[:, 0, :], f4t[:, 0, :], mv4[:, 0:1], r4,
                Alu.subtract, Alu.mult,
            )
            f2bf = work.tile([128, T2, 128], bf16, tag="f2bf", name="f2bf")
            nc.vector.tensor_copy(f2bf, f2t)
            return f2bf, z2, z3, z4

        # stage S2: transposes to channels-first
        def stage2(b, z2, z3, z4):
            z2T = work.tile([128, N2], bf16, tag="z2T", name="z2T")
            transpose_to(z2T, z2, T2, mode="pe")
            z3T = work.tile([128, N3], bf16, tag="z3T", name="z3T")
            transpose_to(z3T, z3, T3, mode="xbar")
            z4T = work.tile([128, N4], bf16, tag="z4T", name="z4T")
            nc.sync.dma_start_transpose(z4T, z4[:, 0, :])
            return z2T, z3T, z4T

        # stage S3: projections, kv, gram, m-matrices
        def stage3(b, z2T, z3T, z4T):
            # ---- q projections + exp (+ free softmax denominators) ----
            eq1 = work2.tile([128, N2], bf16, tag="eq1", name="eq1")
            eq2 = work2.tile([128, N2], bf16, tag="eq2", name="eq2")
            S = small.tile([128, 2, 2], f32, tag="S", name="S")
            for qi, (wq, bqt, eq) in enumerate(((wq1, bq1t, eq1), (wq2, bq2t, eq2))):
                qp0 = psq.tile([128, 512], f32, tag="q", name="qp0")
                qp1 = psq.tile([128, 512], f32, tag="q", name="qp1")
                nc.tensor.matmul(qp0, wq, z2T[:, 0:512], start=True, stop=False)
                nc.tensor.matmul(qp1, wq, z2T[:, 512:1024], start=True, stop=False)
                nc.tensor.matmul(qp0, ident, bqt[:, 0:512], start=False, stop=True)
                nc.tensor.matmul(qp1, ident, bqt[:, 512:1024], start=False, stop=True)
                nc.scalar.activation(eq[:, 0:512], qp0, A.Exp,
                                     accum_out=S[:, qi, 0:1])
                nc.scalar.activation(eq[:, 512:1024], qp1, A.Exp,
                                     accum_out=S[:, qi, 1:2])
            rS = small.tile([128, 2], f32, tag="rS", name="rS")
            nc.vector.tensor_tensor(rS, S[:, :, 0], S[:, :, 1], Alu.add)
            nc.vector.reciprocal(rS, rS)

            # ---- k3/v3 (tokens-first; v-bias preloaded, k-bias via exp table) ----
            kv3p = pssm.tile([128, 512], f32, tag="sm", name="kv3p")
            nc.tensor.matmul(kv3p[:, 256:512], ones1, bv3rep,
                             start=True, stop=False, skip_group_check=True)
            for t in range(T3):
                z3s = z3T[:, t * 128 : (t + 1) * 128]
                nc.tensor.matmul(kv3p[:, t * 128 : (t + 1) * 128], z3s, wk1,
                                 start=True, stop=True, skip_group_check=True)
                nc.tensor.matmul(kv3p[:, 256 + t * 128 : 256 + (t + 1) * 128], z3s,
                                 wv1, start=False, stop=(t == T3 - 1),
                                 skip_group_check=True)
            ek3r = work2.tile([128, T3, 128], bf16, tag="ek3r", name="ek3r")
            nc.scalar.activation(
                ek3r, kv3p[:, 0:256].rearrange("p (t d) -> p t d", t=T3), A.Exp)
            ek3 = work2.tile([128, T3, 128], bf16, tag="ek3", name="ek3")
            nc.vector.tensor_tensor(ek3, ek3r, ebk3, Alu.mult)
            s3 = small.tile([128, T3, 2], f32, tag="s3", name="s3")
            nc.vector.tensor_reduce(
                s3, ek3.rearrange("p t (h e) -> p t h e", h=2), axis=X, op=Alu.add
            )
            nc.vector.reciprocal(s3, s3)
            v3s = work2.tile([128, T3, 128], bf16, tag="v3s", name="v3s")
            nc.vector.tensor_tensor(
                v3s.rearrange("p t (h e) -> p t h e", h=2),
                kv3p[:, 256:512].rearrange("p (t h e) -> p t h e", t=T3, h=2),
                s3.unsqueeze(3).broadcast_to([128, T3, 2, 64]),
                Alu.mult,
            )

            # ---- k4/v4 ----
            kv4p = pssm.tile([N4, 512], f32, tag="sm", name="kv4p")
            nc.tensor.matmul(kv4p[:, 128:256], ones1[:, :N4], bv4rep,
                             start=True, stop=False, skip_group_check=True)
            nc.tensor.matmul(kv4p[:, 0:128], z4T, wk2, start=True, stop=True,
                             skip_group_check=True)
            nc.tensor.matmul(kv4p[:, 128:256], z4T, wv2, start=False, stop=True,
                             skip_group_check=True)
            ek4r = work2.tile([N4, 128], bf16, tag="ek4r", name="ek4r")
            nc.scalar.activation(ek4r, kv4p[:, 0:128], A.Exp)
            ek4 = work2.tile([N4, 128], bf16, tag="ek4", name="ek4")
            nc.vector.tensor_tensor(ek4, ek4r, ebk4, Alu.mult)
            s4 = small.tile([N4, 1, 2], f32, tag="s4", name="s4")
            nc.vector.tensor_reduce(
                s4, ek4.rearrange("p (o h e) -> p o h e", o=1, h=2), axis=X, op=Alu.add
            )
            nc.vector.reciprocal(s4, s4)
            v4s = work2.tile([N4, 128], bf16, tag="v4s", name="v4s")
            nc.vector.tensor_tensor(
                v4s.rearrange("p (h e) -> p h e", h=2),
                kv4p[:, 128:256].rearrange("p (h e) -> p h e", h=2),
                s4[:, 0, :].unsqueeze(2).broadcast_to([N4, 2, 64]),
                Alu.mult,
            )

            # ---- Gram (transposed) + block-diag mask + rp fold ----
            g32p = pssm.tile([128, 128], f32, tag="sm", name="g32p")
            for t in range(T3):
                nc.tensor.matmul(g32p, v3s[:, t, :], ek3[:, t, :],
                                 start=(t == 0), stop=(t == T3 - 1))
            g42p = pssm.tile([128, 128], f32, tag="sm", name="g42p")
            nc.tensor.matmul(g42p, v4s, ek4, start=True, stop=True)

            sl = b % 2
            ms = []
            for gi, (gp, gmt, wrp_, qi) in enumerate(
                ((g32p, gm32, wrp0, 0), (g42p, gm42, wrp1, 1))
            ):
                for h in range(2):
                    nc.vector.tensor_copy(
                        gmt[h * 64 : (h + 1) * 64, sl, h * 64 : (h + 1) * 64],
                        gp[h * 64 : (h + 1) * 64, h * 64 : (h + 1) * 64],
                    )
                mp = pssm.tile([128, 128], f32, tag="sm", name="mp")
                nc.tensor.matmul(mp, gmt[:, sl, :], wrp_,
                                 start=True, stop=True)
                m = work2.tile([128, 128], bf16, tag="m_", name="m_")
                nc.vector.tensor_scalar(m, mp, rS[:, qi : qi + 1], None, Alu.mult)
                ms.append(m)
            return eq1, eq2, ms[0], ms[1]

        # stage S4: attention output, residual, LN4, zx^T
        def stage4(b, f2t, eq1, eq2, m32, m42):
            x_b = x_all[:, b]
            for c in range(2):
                ap_ = psout.tile([128, 512], f32, tag="o", name="ap_")
                nc.tensor.matmul(ap_, ones1, rpbrep, start=True, stop=False,
                                 skip_group_check=True)
                for t in range(4):
                    g = c * 4 + t
                    nc.tensor.matmul(
                        ap_[:, t * 128 : (t + 1) * 128],
                        eq1[:, g * 128 : (g + 1) * 128], m32,
                        start=False, stop=False, skip_group_check=True)
                    nc.tensor.matmul(
                        ap_[:, t * 128 : (t + 1) * 128],
                        eq2[:, g * 128 : (g + 1) * 128], m42,
                        start=False, stop=(t == 3), skip_group_check=True)
                # x = f2 + attn (tokens-first already)
                nc.vector.tensor_tensor(
                    x_b[:, c * 4 : (c + 1) * 4, :].rearrange("p t d -> p (t d)"),
                    f2t[:, c * 4 : (c + 1) * 4, :].rearrange("p t d -> p (t d)"),
                    ap_, Alu.add,
                )

            # ---- LN4 + zx^T ----
            stx, rx, nmx = ln_stats(x_b, T2, 128, "t2x", small)
            zx = work2.tile([128, T2, 128], bf16, tag="zx", name="zx")
            bcast_norm(nc.gpsimd, zx, x_b, rx, nmx, T2)
            transpose_to(zxT_all[:, b * N2 : (b + 1) * N2], zx, T2, mode="mix")

        # pass-2 FFN for one element
        def pass2(b):
            zxT = zxT_all[:, b * N2 : (b + 1) * N2]
            x_b = x_all[:, b]
            out_sb = workp2.tile([128, T2, 128], f32, tag="osb", name="out_sb")
            for c in range(2):
                hp = psout.tile([128, 512], f32, tag="o", name="hp")
                nc.tensor.matmul(hp, ff2bcol, ones512, start=True, stop=False,
                                 skip_group_check=True)
                gps = [None] * 4
                gps[0] = psq.tile([128, 512], f32, tag="q", name="gp")
                nc.tensor.matmul(gps[0], wff1[:, 0:128],
                                 zxT[:, c * 512 : (c + 1) * 512],
                                 start=True, stop=True)
                for j in range(4):
                    gj = workp2.tile([128, 512], bf16, tag="gj", name="gj")
                    nc.scalar.activation(gj, gps[j], A.Gelu, bias=ff1b[:, j : j + 1])
                    if j < 3:
                        gps[j + 1] = psq.tile([128, 512], f32, tag="q", name="gp")
                        nc.tensor.matmul(gps[j + 1],
                                         wff1[:, (j + 1) * 128 : (j + 2) * 128],
                                         zxT[:, c * 512 : (c + 1) * 512],
                                         start=True, stop=True)
                    nc.tensor.matmul(hp, wff2[:, j, :], gj,
                                     start=False, stop=(j == 3),
                                     skip_group_check=True)
                hcf = workp2.tile([128, 512], bf16, tag="hcf", name="hcf")
                nc.vector.tensor_copy(hcf, hp)
                ps = pst.tile([128, 512], bf16, tag="pst", name="psh")
                for t in range(4):
                    nc.tensor.transpose(
                        ps[:, t * 128 : (t + 1) * 128],
                        hcf[:, t * 128 : (t + 1) * 128], ident)
                nc.vector.tensor_tensor(
                    out_sb[:, c * 4 : (c + 1) * 4, :].rearrange("p t d -> p (t d)"),
                    x_b[:, c * 4 : (c + 1) * 4, :].rearrange("p t d -> p (t d)"),
                    ps, Alu.add,
                )
            nc.sync.dma_start(outd[b].rearrange("(t p) d -> p t d", p=128), out_sb)

        # ---------------- pipelined emission ----------------
        # pass-1 stages with a 3-deep skew; pass-2 emitted in 4-element
        # batches as soon as its inputs are complete, so the scheduler can
        # fill PE gaps (and keep HAM warm) with dense FFN matmuls while
        # bounding ACT-table swaps.
        npair = nb // 2
        s1o = {}
        s12 = {}
        s3o = {}
        for i in range(npair + 2):
            if i < npair:
                for b in (2 * i, 2 * i + 1):
                    s1o[b] = stage1(b)
            if 0 <= i - 1 < npair:
                for b in (2 * (i - 1), 2 * (i - 1) + 1):
                    s3o[b] = stage3(b, *s12[b][1])
            if 0 <= i - 2 < npair:
                for b in (2 * (i - 2), 2 * (i - 2) + 1):
                    stage4(b, s12[b][0], *s3o.pop(b))
                    del s12[b]
            if i < npair:
                for b in (2 * i, 2 * i + 1):
                    f2bf, z2, z3, z4 = s1o.pop(b)
                    s12[b] = (f2bf, stage2(b, z2, z3, z4))
        for b2 in range(nb):
            pass2(b2)

    nc.compile()
    return nc


def _get_program():
    global _PROGRAM
    if _PROGRAM is None:
        _PROGRAM = _build_program(NB)
    return _PROGRAM


def _prepare_params(inputs):
    bf = ml_dtypes.bfloat16
    g = {k: np.asarray(v, np.float32) for k, v in inputs.items()
         if k not in ("f2", "f3", "f4")}
    pe2, pe3, pe4 = g["pe2"][0], g["pe3"][0], g["pe4"][0]

    def fold_w(ln_w, w):
        return np.ascontiguousarray(ln_w[:, None] * w).astype(bf)

    def fold_bt(ln_b, pe, w, b):
        return np.ascontiguousarray(((ln_b[None, :] + pe) @ w + b[None, :]).T).astype(bf)

    p = {}
    p["wq1"] = fold_w(g["ln1_w"], g["q1_w"])
    p["wq2"] = fold_w(g["ln1_w"], g["q2_w"])
    p["wk1"] = fold_w(g["ln2_w"], g["k1_w"])
    p["wv1"] = fold_w(g["ln2_w"], g["v1_w"])
    p["wk2"] = fold_w(g["ln3_w"], g["k2_w"])
    p["wv2"] = fold_w(g["ln3_w"], g["v2_w"])
    p["bq1t"] = fold_bt(g["ln1_b"], pe2, g["q1_w"], g["q1_b"])
    p["bq2t"] = fold_bt(g["ln1_b"], pe2, g["q2_w"], g["q2_b"])
    bk3 = (g["ln2_b"][None, :] + pe3) @ g["k1_w"] + g["k1_b"][None, :]
    bk4 = (g["ln3_b"][None, :] + pe4) @ g["k2_w"] + g["k2_b"][None, :]
    p["ebk3"] = np.ascontiguousarray(
        np.exp(bk3).reshape(T3, 128, 128).transpose(1, 0, 2)).astype(bf)
    p["ebk4"] = np.ascontiguousarray(np.exp(bk4)).astype(bf)
    bv3row = g["ln2_b"] @ g["v1_w"] + g["v1_b"]
    bv4row = g["ln3_b"] @ g["v2_w"] + g["v2_b"]
    p["bv3rep"] = np.ascontiguousarray(np.tile(bv3row, 2)[None, :]).astype(bf)
    p["bv4rep"] = np.ascontiguousarray(bv4row[None, :]).astype(bf)
    p["rpbrep"] = np.ascontiguousarray(np.tile(g["rp_b"], 4)[None, :]).astype(bf)
    p["wrp"] = np.ascontiguousarray(g["rp_w"].reshape(2, D, D)).astype(bf)
    p["wff1"] = fold_w(g["ln4_w"], g["ff1_w"])
    bff1 = g["ln4_b"] @ g["ff1_w"] + g["ff1_b"]
    p["ff1b"] = np.ascontiguousarray(bff1.reshape(4, 128).T).astype(np.float32)
    p["wff2"] = np.ascontiguousarray(g["ff2_w"].reshape(4, 128, D)).astype(bf)
    p["ff2bcol"] = np.ascontiguousarray(g["ff2_b"][None, :]).astype(bf)
    p["ones512"] = np.ones((1, 512), np.float32).astype(bf)
    p["ones1"] = np.ones((1, 128), np.float32).astype(bf)
    p["ident"] = np.eye(128, dtype=np.float32).astype(bf)
    return p


def kernel(**inputs):
    global LAST_RESULTS
    from concourse import bass_utils

    f2 = np.ascontiguousarray(np.asarray(inputs["f2"], np.float32))
    f3 = np.ascontiguousarray(np.asarray(inputs["f3"], np.float32))
    f4 = np.ascontiguousarray(np.asarray(inputs["f4"], np.float32))
    params = _prepare_params(inputs)
    nc = _get_program()

    in_maps = []
    for c in range(NCORES):
        m = dict(params)
        sl = slice(c * NB, (c + 1) * NB)
        m["f2"] = f2[sl]
        m["f3"] = f3[sl]
        m["f4"] = f4[sl]
        in_maps.append(m)

    res = bass_utils.run_bass_kernel_spmd(
        nc, in_maps, list(range(NCORES)),
        trace=bool(int(os.environ.get("KERNEL_TRACE", "0"))),
    )
    LAST_RESULTS = res
    out = np.concatenate([r["out"] for r in res.results], axis=0)
    return np.ascontiguousarray(out.astype(np.float32))
